# revision 15
# baseline (speedup 1.0000x reference)
"""Tensor-parallel GQA attention block (dense_transformer) on 8 TRN2 NeuronCores.

Sharding: tensor parallel across heads — core c owns q-heads 4c..4c+3 and
kv-head c (GQA groups intact). Instead of a row-parallel wo + AllReduce,
each core AllGathers the softmax-normalized per-head attention outputs y
(bf16, 4 pair-level collectives) and computes a 512-column slice of the
output projection; the host concatenates slices.

Device-side design (v2):
  - all matmul operands live "contraction dim on partitions": xT [DIM,S],
    wqkvT (fused q|k|v) [DIM,768], woT [DIM,512]; scores are computed
    transposed (S^T = K-tile^T @ Q^T) so no on-chip transposes of P are
    needed; V is produced directly in [s,hd] layout by swapping the
    stationary operand (xs becomes the weights) — no PE transposes.
  - RoPE reads the projection PSUM directly on the DVE (no scalar-copy
    eviction chain); psum reads are ordered to free banks in the order
    the next chunk's matmuls need them, and the q01 psum tag alternates
    between two banks on chunk parity so back-to-back chunks never WAR.
  - attention runs per head-PAIR with a 1-tile software pipeline:
    scores(t) for both heads share one K ldweights, psy(t-1) for both
    heads share one V ldweights; the PE stream is pure N=512 matmuls.
  - softmax denominators come from a running bf16 accumulation of the
    exp tiles on the DVE (one tensor_add per tile) finished by a single
    ones-matmul per (head, half) — this removes the 128x1x512 matmul per
    (head, tile) of the previous version (~55us of PE time).
  - AllGathers are split per head-pair (4 total) so transfers start at
    the midpoint of each attention phase; gather tiles are prefetched
    into SBUF through staged pool releases so outproj never waits.
  - compute dtype: bf16 matmul operands (fp32 PSUM accumulation), fp32
    RoPE/softmax arithmetic, bf16 denominator accumulation.
"""

import ml_dtypes
import numpy as np

import concourse.bass as bass
import concourse.mybir as mybir
import concourse.tile as tile
from concourse import bacc
from concourse.bass_utils import run_bass_kernel_spmd

F32 = mybir.dt.float32
BF16 = mybir.dt.bfloat16
AF = mybir.ActivationFunctionType

N_CORES = 8
DIM = 4096
S = 2048
HEAD_DIM = 128
N_HEADS = 32
N_KV = 8
HPC = N_HEADS // N_CORES        # q heads per core = 4
P = 128
SC = 512                        # seq chunk (free dim of most matmuls)
IC = 1024                       # attention i-chunk (2 seq chunks)
N_SCHUNK = S // SC              # 4
N_KTILE = DIM // P              # 32
N_STILE = S // P                # 16

SWAP16 = list(range(16, 32)) + list(range(16))   # per-quadrant 16-rotation


def build(debug_taps: bool = False):
    nc = bacc.Bacc(None, num_devices=N_CORES)

    xT = nc.declare_dram_parameter("xT", [DIM, S], BF16, isOutput=False)
    # fused qkv weights: [:, 0:512] q heads, [:, 512:640] k, [:, 640:768] v
    wqkvT = nc.declare_dram_parameter("wqkvT", [DIM, 768], BF16, isOutput=False)
    woT = nc.declare_dram_parameter("woT", [DIM, SC], BF16, isOutput=False)
    cosd = nc.declare_dram_parameter("cosd", [P, S], F32, isOutput=False)
    sins = nc.declare_dram_parameter("sins", [P, S], F32, isOutput=False)
    # causal band image: cmask[j, v] = 0 if (v - IC) >= j else -1e9
    cmask = nc.declare_dram_parameter("cmask", [P, 2 * IC], F32, isOutput=False)
    out = nc.dram_tensor("out", [S, SC], F32, kind="ExternalOutput")

    taps = {}
    if debug_taps:
        taps["qt"] = nc.dram_tensor("qt", [P, HPC, S], F32, kind="ExternalOutput")
        taps["kt"] = nc.dram_tensor("kt", [P, S], F32, kind="ExternalOutput")
        taps["vv"] = nc.dram_tensor("vv", [P, N_STILE, HEAD_DIM], F32, kind="ExternalOutput")
        taps["dd"] = nc.dram_tensor("dd", [HPC, S], F32, kind="ExternalOutput")
        taps["yl"] = nc.dram_tensor("yl", [P, HPC, S], BF16, kind="ExternalOutput")

    with tile.TileContext(nc) as tc:
        # PSUM tags (8 banks total, 4KB/partition each tag):
        #   proj:    q01 -> A (even chunks) / D (odd chunks), q23 -> B,
        #            k+v -> C ([:,0,:] = kT, [:,1,:] reshaped = v blocks)
        #   attn:    A/B = scores h-even/h-odd, C/D = psy h-even/h-odd,
        #            drain reuses A..D for the [1,SC] denominator matmuls
        #   outproj: C/D alternate for pso
        ps = tc.alloc_tile_pool(name="ps", bufs=1, space="PSUM")
        const = tc.alloc_tile_pool(name="const", bufs=1)
        pw2 = tc.alloc_tile_pool(name="pw2", bufs=1, side="right")
        pw = tc.alloc_tile_pool(name="pw", bufs=1, side="right")
        main = tc.alloc_tile_pool(name="main", bufs=1)
        stream = tc.alloc_tile_pool(name="stream", bufs=3)
        tmp = tc.alloc_tile_pool(name="tmp", bufs=2)
        dram = tc.alloc_tile_pool(name="dram", bufs=1, space="DRAM")

        # ---- weights first (they gate the first matmuls) ------------------
        wqkv_sb = pw.tile([P, N_KTILE, 768], BF16)
        for k in range(N_KTILE):
            nc.scalar.dma_start(wqkv_sb[:, k, :], wqkvT[k * P:(k + 1) * P, :])

        ones_f = const.tile([P, P], F32)
        nc.vector.memset(ones_f[:], 1.0)
        ones = const.tile([P, P], BF16)
        nc.scalar.copy(ones[:], ones_f[:])
        mask_sb = const.tile([P, 2 * IC], F32)
        nc.gpsimd.dma_start(mask_sb[:], cmask[:])
        cos_sb = pw.tile([P, S], F32)
        sin_sb = pw.tile([P, S], F32)
        nc.gpsimd.dma_start(cos_sb[:], cosd[:])
        nc.gpsimd.dma_start(sin_sb[:], sins[:])
        wo_sb = pw2.tile([P, N_KTILE, SC], BF16)

        kt_sb = main.tile([P, S], BF16)
        v_sb = main.tile([P, N_STILE, HEAD_DIM], BF16)
        qt_sb = main.tile([P, HPC, S], BF16)

        # per (cp, pair) bounce + gather buffers
        ybounce = [
            [dram.tile([2 * P, IC], BF16, name=f"ybounce{cp}_{pr}")
             for pr in range(2)]
            for cp in range(2)
        ]
        ygather = [
            [dram.tile([N_CORES * 2 * P, IC], BF16, addr_space="Shared",
                       name=f"ygather{cp}_{pr}")
             for pr in range(2)]
            for cp in range(2)
        ]
        yg_tiles = {}

        # ---- projection: one 512-seq chunk ------------------------------
        def proj(ci):
            s_lo = ci * SC
            qtag = "A" if ci % 2 == 0 else "D"
            psq01 = ps.tile([P, 2, SC], F32, tag=qtag, name=f"psq01_{ci}")
            psq23 = ps.tile([P, 2, SC], F32, tag="B", name=f"psq23_{ci}")
            pskv = ps.tile([P, 2, SC], F32, tag="C", name=f"pskv_{ci}")
            psv = pskv[:, 1, :].rearrange("p (b d) -> p b d", b=4)
            for k in range(N_KTILE):
                xs = stream.tile([P, SC], BF16, tag="xs", bufs=12, name=f"xs{ci}_{k}")
                nc.sync.dma_start(xs[:], xT[k * P:(k + 1) * P, s_lo:s_lo + SC])
                st = dict(start=(k == 0), stop=(k == N_KTILE - 1))
                for h in range(HPC):
                    dst = (psq01 if h < 2 else psq23)[:, h % 2, :]
                    nc.tensor.matmul(
                        dst, wqkv_sb[:, k, h * P:(h + 1) * P], xs[:], **st
                    )
                nc.tensor.matmul(pskv[:, 0, :], wqkv_sb[:, k, 512:640], xs[:], **st)
                # v via stationary swap: out[s_block, d] accumulated over k.
                # All 4 s-blocks share ONE psum bank; start=True clears
                # has_written for the whole bank, so only the very first
                # matmul may carry it — later blocks' first writes overwrite
                # (bit clear) then accumulate, which is exactly right.
                for sb in range(4):
                    nc.tensor.matmul(
                        psv[:, sb, :],
                        xs[:, sb * P:(sb + 1) * P],
                        wqkv_sb[:, k, 640:768],
                        start=(k == 0 and sb == 0),
                        stop=(k == N_KTILE - 1),
                        skip_group_check=True,
                    )

            # RoPE directly from PSUM on the DVE. Bank-freeing order matches
            # the next chunk's write order: B (q23), C (k, then v), then the
            # q01 tag (not needed until chunk+2 thanks to the A/D rotation).
            qcs, qsws = {}, {}

            def rope_reads(idx, src):
                qc = tmp.tile([P, SC], F32, tag="rqc", bufs=3, name=f"rq{ci}_{idx}")
                nc.vector.tensor_mul(qc[:], src, cos_sb[:, s_lo:s_lo + SC])
                qsw = tmp.tile([P, SC], F32, tag="rqs", bufs=3, name=f"rs{ci}_{idx}")
                nc.vector.stream_shuffle(qsw[:], src, SWAP16)
                qcs[idx], qsws[idx] = qc, qsw

            def rope_tail(idx):
                dst = kt_sb[:, s_lo:s_lo + SC] if idx == 4 \
                    else qt_sb[:, idx, s_lo:s_lo + SC]
                nc.vector.tensor_mul(qsws[idx][:], qsws[idx][:],
                                     sin_sb[:, s_lo:s_lo + SC])
                nc.vector.tensor_add(dst, qcs[idx][:], qsws[idx][:])

            rope_reads(2, psq23[:, 0, :])
            rope_reads(3, psq23[:, 1, :])
            rope_reads(4, pskv[:, 0, :])
            nc.vector.tensor_copy(v_sb[:, ci * 4:(ci + 1) * 4, :], psv[:, :, :])
            rope_tail(2)
            rope_tail(3)
            rope_tail(4)
            rope_reads(0, psq01[:, 0, :])
            rope_reads(1, psq01[:, 1, :])
            rope_tail(0)
            rope_tail(1)

        # ---- attention: one i-chunk-pair, head pairs pipelined ----------
        def attn(cp):
            n_j = 8 * (cp + 1)
            state = {}          # per pair: pss/pt/acc handles

            def u_list(t):
                return [u for u in (0, 1) if t < 8 * cp + 4 + 4 * u]

            def emit_scores(pr, t):
                ul = u_list(t)
                sd = state.setdefault(pr, {"pss": {}, "pt": {}, "acc": {}})
                for hh in range(2):
                    h = 2 * pr + hh
                    pss = ps.tile([P, IC], F32, tag=("A" if hh == 0 else "B"),
                                  name=f"pss{h}_{cp}_{t}")
                    sd["pss"][hh] = pss
                    for u in ul:
                        nc.tensor.matmul(
                            pss[:, u * SC:(u + 1) * SC],
                            kt_sb[:, t * P:(t + 1) * P],
                            qt_sb[:, h, cp * IC + u * SC:cp * IC + (u + 1) * SC],
                            start=True, stop=True,
                        )
                for hh in range(2):
                    h = 2 * pr + hh
                    pss = sd["pss"][hh]
                    pt = tmp.tile([P, IC], BF16, tag=f"pt{hh}", bufs=2,
                                  name=f"pt{h}_{cp}_{t}")
                    sd["pt"][(t, hh)] = pt
                    d = t - 8 * cp
                    if d < 0:
                        nc.scalar.activation(pt[:], pss[:], AF.Exp)
                    else:
                        for u in ul:
                            sl = slice(u * SC, (u + 1) * SC)
                            if t <= 8 * cp + 4 * u - 1:
                                nc.scalar.activation(pt[:, sl], pss[:, sl], AF.Exp)
                            else:
                                ms = tmp.tile([P, SC], F32, tag="ms", bufs=2,
                                              name=f"ms{h}_{cp}_{t}_{u}")
                                nc.vector.tensor_add(
                                    ms[:], pss[:, sl],
                                    mask_sb[:, IC - P * d + u * SC:
                                            2 * IC - P * d + (u - 1) * SC],
                                )
                                nc.scalar.activation(pt[:, sl], ms[:], AF.Exp)
                    # running bf16 accumulation of exp tiles (denominator)
                    if t == 0:
                        acc = tmp.tile([P, IC], BF16, tag=f"acc{hh}", bufs=2,
                                       name=f"acc{h}_{cp}")
                        nc.vector.tensor_copy(acc[:], pt[:])
                        sd["acc"][hh] = acc
                    else:
                        acc = sd["acc"][hh]
                        if len(ul) == 2:
                            nc.vector.tensor_add(acc[:], acc[:], pt[:])
                        else:
                            sl = slice(ul[0] * SC, (ul[0] + 1) * SC)
                            nc.vector.tensor_add(acc[:, sl], acc[:, sl], pt[:, sl])

            def emit_psy(pr, t):
                ul = u_list(t)
                sd = state[pr]
                for hh in range(2):
                    h = 2 * pr + hh
                    if t == 0:
                        sd[f"psy{hh}"] = ps.tile(
                            [P, IC], F32, tag=("C" if hh == 0 else "D"),
                            name=f"psy{h}_{cp}",
                        )
                    psy = sd[f"psy{hh}"]
                    pt = sd["pt"].pop((t, hh))
                    for u in ul:
                        sl = slice(u * SC, (u + 1) * SC)
                        nc.tensor.matmul(
                            psy[:, sl], v_sb[:, t, :], pt[:, sl],
                            start=(t == 0), stop=(t == 8 * cp + 3 + 4 * u),
                        )

            def emit_drain(pr):
                sd = state[pr]
                # evict psy fast so the next pair's scores can retake banks
                ysbs = {}
                for hh in range(2):
                    ysb = tmp.tile([P, IC], F32, tag=f"ysb{hh}", bufs=1,
                                   name=f"ysb{2*pr+hh}_{cp}")
                    nc.vector.tensor_copy(ysb[:], sd[f"psy{hh}"][:])
                    ysbs[hh] = ysb
                # denominator: one ones-matmul per (head, half); reciprocal
                # reads the psum directly (no SBUF staging)
                dtags = [("A", 0, 0), ("B", 0, 1), ("C", 1, 0), ("D", 1, 1)]
                rc1s = [
                    tmp.tile([1, IC], F32, tag=f"rc1{hh}", bufs=1,
                             name=f"rc1{pr}_{cp}_{hh}")
                    for hh in range(2)
                ]
                for tg, hh, u in dtags:
                    psd = ps.tile([1, SC], F32, tag=tg,
                                  name=f"psd{pr}_{cp}_{hh}_{u}")
                    nc.tensor.matmul(
                        psd[:], ones[:, 0:1],
                        sd["acc"][hh][:, u * SC:(u + 1) * SC],
                        start=True, stop=True,
                    )
                    nc.vector.reciprocal_approx_fast(
                        rc1s[hh][:, u * SC:(u + 1) * SC], psd[:]
                    )
                for hh in range(2):
                    h = 2 * pr + hh
                    rc1 = rc1s[hh]
                    rbb = tmp.tile([P, IC], F32, tag=f"rbb{hh}", bufs=1,
                                   name=f"rbb{h}_{cp}")
                    nc.gpsimd.partition_broadcast(rbb[:], rc1[:])
                    yp = tmp.tile([P, IC], BF16, tag=f"yp{hh}", bufs=1,
                                  name=f"yp{h}_{cp}")
                    nc.vector.tensor_mul(yp[:], ysbs[hh][:], rbb[:])
                    nc.gpsimd.dma_start(
                        ybounce[cp][pr][hh * P:(hh + 1) * P, :], yp[:]
                    )
                    if debug_taps:
                        s_lo = cp * IC
                        nc.sync.dma_start(taps["yl"][:, h, s_lo:s_lo + IC], yp[:])
                        # dd tap holds 1/D (reciprocal) in this version
                        nc.sync.dma_start(
                            taps["dd"][h:h + 1, s_lo:s_lo + IC], rc1[:]
                        )
                nc.gpsimd.collective_compute(
                    "AllGather",
                    mybir.AluOpType.bypass,
                    replica_groups=[list(range(N_CORES))],
                    ins=[ybounce[cp][pr][:]],
                    outs=[ygather[cp][pr][:]],
                )
                del state[pr]

            items = [(pr, t) for pr in range(2) for t in range(n_j)]
            for i in range(len(items) + 1):
                if i < len(items):
                    emit_scores(*items[i])
                if i > 0:
                    pr2, t2 = items[i - 1]
                    emit_psy(pr2, t2)
                    if t2 == n_j - 1:
                        emit_drain(pr2)

        # ---- yg prefetch + output projection ----------------------------
        def load_yg(cp, pool):
            for pr in range(2):
                yg = pool.tile([P, 16, IC], BF16, tag=f"yg{cp}_{pr}",
                               name=f"yg{cp}_{pr}")
                src = ygather[cp][pr][:].rearrange("(b p) m -> p b m", p=P)
                nc.sync.dma_start(yg[:, 0:8, :], src[:, 0:8, :])
                nc.scalar.dma_start(yg[:, 8:16, :], src[:, 8:16, :])
                yg_tiles[(cp, pr)] = yg

        def outproj(ci):
            g_lo = ci * SC
            cp, u = ci // 2, ci % 2
            for st_i in range(4):
                pso = ps.tile(
                    [P, SC], F32, tag=("C" if st_i % 2 == 0 else "D"),
                    name=f"pso{ci}_{st_i}",
                )
                for kt in range(N_KTILE):
                    c, h = kt // 4, kt % 4
                    src_t = yg_tiles[(cp, h // 2)]
                    nc.tensor.matmul(
                        pso[:],
                        src_t[:, 2 * c + h % 2,
                              u * SC + st_i * P:u * SC + (st_i + 1) * P],
                        wo_sb[:, kt, :],
                        start=(kt == 0), stop=(kt == N_KTILE - 1),
                    )
                ob = fin.tile([P, SC], F32, tag="ob", bufs=2,
                              name=f"ob{ci}_{st_i}")
                nc.scalar.copy(ob[:], pso[:])
                nc.gpsimd.dma_start(
                    out[g_lo + st_i * P:g_lo + (st_i + 1) * P, :], ob[:]
                )

        # ---- pipeline ----------------------------------------------------
        proj(0)
        proj(1)
        attn(0)          # -> AG pair 0a, 0b
        nc.scalar.dma_start(wo_sb[:], woT.rearrange("(t p) m -> p t m", p=P))
        proj(2)
        proj(3)
        pw.release()
        pyg0 = tc.alloc_tile_pool(name="pyg0", bufs=1, side="right")
        load_yg(0, pyg0)     # runs during attn(1); AG0 long done
        attn(1)          # -> AG pair 1a, 1b

        if debug_taps:
            nc.gpsimd.dma_start(taps["qt"][:], qt_sb[:])
            nc.gpsimd.dma_start(taps["kt"][:], kt_sb[:])
            nc.gpsimd.dma_start(taps["vv"][:], v_sb[:])

        tmp.release()
        stream.release()
        main.release()
        const.release()
        pyg1 = tc.alloc_tile_pool(name="pyg1", bufs=1, side="right")
        fin = tc.alloc_tile_pool(name="fin", bufs=1, side="right")
        load_yg(1, pyg1)     # waits on AG1a/b, runs during outproj(0..1)
        outproj(0)
        outproj(1)
        outproj(2)
        outproj(3)

        for pool in (fin, pyg1, pyg0, pw2, dram, ps):
            pool.release()

    nc.compile()
    return nc


# ---------------------------------------------------------------------------
# host-side prep / unshard
# ---------------------------------------------------------------------------

def _perm128():
    """head-dim permutation: pair i=(16q+j) -> even at 32q+j, odd at 32q+16+j."""
    order = np.empty(128, dtype=np.int64)
    for i in range(64):
        q, j = i // 16, i % 16
        order[32 * q + j] = 2 * i
        order[32 * q + 16 + j] = 2 * i + 1
    return order


def _host_prep(x, freqs_cis, wq, wk, wv, wo):
    order = _perm128()
    xT = np.ascontiguousarray(x[0].T)                       # [DIM, S]
    scale = np.float32(1.0 / np.sqrt(HEAD_DIM))

    cosT = np.ascontiguousarray(freqs_cis[:, :, 0].T)       # [64, S]
    sinT = np.ascontiguousarray(freqs_cis[:, :, 1].T)
    cosd = np.empty((P, S), dtype=np.float32)
    sins = np.empty((P, S), dtype=np.float32)
    for q in range(4):
        cosd[32 * q:32 * q + 16] = cosT[16 * q:16 * q + 16]
        cosd[32 * q + 16:32 * q + 32] = cosT[16 * q:16 * q + 16]
        sins[32 * q:32 * q + 16] = -sinT[16 * q:16 * q + 16]
        sins[32 * q + 16:32 * q + 32] = sinT[16 * q:16 * q + 16]

    vv = np.arange(2 * IC)[None, :]
    jj = np.arange(P)[:, None]
    cmask = np.where(vv - IC >= jj, np.float32(0.0), np.float32(-1e9))
    cmask = np.ascontiguousarray(cmask, dtype=np.float32)

    xT16 = xT.astype(ml_dtypes.bfloat16)
    in_maps = []
    for c in range(N_CORES):
        wq_c = wq[c * 512:(c + 1) * 512].reshape(HPC, 128, DIM)[:, order, :]
        wq_c = (wq_c.reshape(512, DIM) * scale).astype(np.float32)
        wk_c = wk[c * 128:(c + 1) * 128][order]
        wv_c = wv[c * 128:(c + 1) * 128]
        wqkv_c = np.concatenate([wq_c, wk_c, wv_c], axis=0)
        wo_c = wo[c * 512:(c + 1) * 512]
        in_maps.append({
            "xT": xT16,
            "wqkvT": np.ascontiguousarray(wqkv_c.T).astype(ml_dtypes.bfloat16),
            "woT": np.ascontiguousarray(wo_c.T).astype(ml_dtypes.bfloat16),
            "cosd": cosd,
            "sins": sins,
            "cmask": cmask,
        })
    return in_maps


_NC_CACHE = {}


def get_nc(debug_taps=False):
    key = bool(debug_taps)
    if key not in _NC_CACHE:
        _NC_CACHE[key] = build(debug_taps=key)
    return _NC_CACHE[key]


def kernel(x, freqs_cis, mask, wq, wk, wv, wo, _trace=False, _debug_taps=False,
           _warmup=False):
    in_maps = _host_prep(x, freqs_cis, wq, wk, wv, wo)
    nc = get_nc(_debug_taps)
    if _warmup:
        run_bass_kernel_spmd(
            nc, in_maps, core_ids=list(range(N_CORES)), trace=False
        )
    res = run_bass_kernel_spmd(
        nc, in_maps, core_ids=list(range(N_CORES)), trace=_trace
    )
    full = np.concatenate([res.results[c]["out"] for c in range(N_CORES)], axis=1)
    out = full.reshape(1, S, DIM).astype(np.float32)
    if _trace or _debug_taps:
        kernel.last_results = res
    return out


# revision 26
# speedup vs baseline: 1.1141x; 1.1141x over previous
"""Tensor-parallel GQA attention block (dense_transformer) on 8 TRN2 NeuronCores.

Sharding: tensor parallel across heads — core c owns q-heads 4c..4c+3 and
kv-head c (GQA groups intact). Instead of a row-parallel wo + AllReduce,
each core AllGathers the softmax-normalized per-head attention outputs y
(bf16, 4 pair-level collectives) and computes a 512-column slice of the
output projection; the host concatenates slices.

Device-side design (v2):
  - all matmul operands live "contraction dim on partitions": xT [DIM,S],
    wqkvT (fused q|k|v) [DIM,768], woT [DIM,512]; scores are computed
    transposed (S^T = K-tile^T @ Q^T) so no on-chip transposes of P are
    needed; V is produced directly in [s,hd] layout by swapping the
    stationary operand (xs becomes the weights) — no PE transposes.
  - RoPE reads the projection PSUM directly on the DVE (no scalar-copy
    eviction chain); psum reads are ordered to free banks in the order
    the next chunk's matmuls need them, and the q01 psum tag alternates
    between two banks on chunk parity so back-to-back chunks never WAR.
  - attention runs per head-PAIR with a 1-tile software pipeline:
    scores(t) for both heads share one K ldweights, psy(t-1) for both
    heads share one V ldweights; the PE stream is pure N=512 matmuls.
  - softmax denominators come from a running bf16 accumulation of the
    exp tiles on the DVE (one tensor_add per tile) finished by a single
    ones-matmul per (head, half) — this removes the 128x1x512 matmul per
    (head, tile) of the previous version (~55us of PE time).
  - AllGathers are split per head-pair (4 total) so transfers start at
    the midpoint of each attention phase; gather tiles are prefetched
    into SBUF through staged pool releases so outproj never waits.
  - compute dtype: bf16 matmul operands (fp32 PSUM accumulation), fp32
    RoPE/softmax arithmetic, bf16 denominator accumulation.
"""

import ml_dtypes
import numpy as np

import concourse.bass as bass
import concourse.mybir as mybir
import concourse.tile as tile
from concourse import bacc
from concourse.bass_utils import run_bass_kernel_spmd

F32 = mybir.dt.float32
BF16 = mybir.dt.bfloat16
AF = mybir.ActivationFunctionType

N_CORES = 8
DIM = 4096
S = 2048
HEAD_DIM = 128
N_HEADS = 32
N_KV = 8
HPC = N_HEADS // N_CORES        # q heads per core = 4
P = 128
SC = 512                        # seq chunk (free dim of most matmuls)
IC = 1024                       # attention i-chunk (2 seq chunks)
N_SCHUNK = S // SC              # 4
N_KTILE = DIM // P              # 32
N_STILE = S // P                # 16

SWAP16 = list(range(16, 32)) + list(range(16))   # per-quadrant 16-rotation


def build(debug_taps: bool = False):
    nc = bacc.Bacc(None, num_devices=N_CORES)

    xT = nc.declare_dram_parameter("xT", [DIM, S], BF16, isOutput=False)
    # fused qkv weights: [:, 0:512] q heads, [:, 512:640] k, [:, 640:768] v
    wqkvT = nc.declare_dram_parameter("wqkvT", [DIM, 768], BF16, isOutput=False)
    woT = nc.declare_dram_parameter("woT", [DIM, SC], BF16, isOutput=False)
    cosd = nc.declare_dram_parameter("cosd", [P, S], BF16, isOutput=False)
    sins = nc.declare_dram_parameter("sins", [P, S], BF16, isOutput=False)
    # causal band image: cmask[j, v] = 0 if (v - IC) >= j else -1e9
    cmask = nc.declare_dram_parameter("cmask", [P, 2 * IC], F32, isOutput=False)
    out = nc.dram_tensor("out", [S, SC], F32, kind="ExternalOutput")

    taps = {}
    if debug_taps:
        taps["qt"] = nc.dram_tensor("qt", [P, HPC, S], F32, kind="ExternalOutput")
        taps["kt"] = nc.dram_tensor("kt", [P, S], F32, kind="ExternalOutput")
        taps["vv"] = nc.dram_tensor("vv", [P, N_STILE, HEAD_DIM], F32, kind="ExternalOutput")
        taps["dd"] = nc.dram_tensor("dd", [HPC, S], F32, kind="ExternalOutput")
        taps["yl"] = nc.dram_tensor("yl", [P, HPC, S], BF16, kind="ExternalOutput")

    with tile.TileContext(nc) as tc:
        # PSUM tags (8 banks total, 4KB/partition each tag):
        #   proj:    q01 -> A (even chunks) / D (odd chunks), q23 -> B,
        #            k+v -> C ([:,0,:] = kT, [:,1,:] reshaped = v blocks)
        #   attn:    A/B = scores h-even/h-odd, C/D = psy h-even/h-odd,
        #            drain reuses A..D for the [1,SC] denominator matmuls
        #   outproj: C/D alternate for pso
        ps = tc.alloc_tile_pool(name="ps", bufs=1, space="PSUM")
        const = tc.alloc_tile_pool(name="const", bufs=1)
        pw2 = tc.alloc_tile_pool(name="pw2", bufs=1, side="right")
        pw = tc.alloc_tile_pool(name="pw", bufs=1, side="right")
        main = tc.alloc_tile_pool(name="main", bufs=1)
        stream = tc.alloc_tile_pool(name="stream", bufs=3)
        tmp = tc.alloc_tile_pool(name="tmp", bufs=2)
        dram = tc.alloc_tile_pool(name="dram", bufs=1, space="DRAM")

        # ---- weights first (they gate the first matmuls) ------------------
        wqkv_sb = pw.tile([P, N_KTILE, 768], BF16)
        for k in range(N_KTILE):
            nc.scalar.dma_start(wqkv_sb[:, k, :], wqkvT[k * P:(k + 1) * P, :])

        ones_f = const.tile([P, P], F32)
        nc.vector.memset(ones_f[:], 1.0)
        ones = const.tile([P, P], BF16)
        nc.scalar.copy(ones[:], ones_f[:])
        mask_sb = const.tile([P, 2 * IC], F32)
        nc.gpsimd.dma_start(mask_sb[:], cmask[:])
        cos_sb = pw.tile([P, S], BF16)
        sin_sb = pw.tile([P, S], BF16)
        nc.gpsimd.dma_start(cos_sb[:], cosd[:])
        nc.gpsimd.dma_start(sin_sb[:], sins[:])
        wo_sb = pw2.tile([P, N_KTILE, SC], BF16)

        kt_sb = main.tile([P, S], BF16)
        v_sb = main.tile([P, N_STILE, HEAD_DIM], BF16)
        qt_sb = main.tile([P, HPC, S], BF16)

        # per (cp, pair) bounce + gather buffers
        ybounce = [
            [dram.tile([2 * P, IC], BF16, name=f"ybounce{cp}_{pr}")
             for pr in range(2)]
            for cp in range(2)
        ]
        ygather = [
            [dram.tile([N_CORES * 2 * P, IC], BF16, addr_space="Shared",
                       name=f"ygather{cp}_{pr}")
             for pr in range(2)]
            for cp in range(2)
        ]
        yg_tiles = {}

        # ---- projection: one 512-seq chunk ------------------------------
        def proj(ci):
            s_lo = ci * SC
            qtag = "A" if ci % 2 == 0 else "D"
            psq01 = ps.tile([P, 2, SC], F32, tag=qtag, name=f"psq01_{ci}")
            psq23 = ps.tile([P, 2, SC], F32, tag="B", name=f"psq23_{ci}")
            pskv = ps.tile([P, 2, SC], F32, tag="C", name=f"pskv_{ci}")
            psv = pskv[:, 1, :].rearrange("p (b d) -> p b d", b=4)
            for k in range(N_KTILE):
                xs = stream.tile([P, SC], BF16, tag="xs", bufs=12, name=f"xs{ci}_{k}")
                nc.sync.dma_start(xs[:], xT[k * P:(k + 1) * P, s_lo:s_lo + SC])
                st = dict(start=(k == 0), stop=(k == N_KTILE - 1))
                for h in range(HPC):
                    dst = (psq01 if h < 2 else psq23)[:, h % 2, :]
                    nc.tensor.matmul(
                        dst, wqkv_sb[:, k, h * P:(h + 1) * P], xs[:], **st
                    )
                nc.tensor.matmul(pskv[:, 0, :], wqkv_sb[:, k, 512:640], xs[:], **st)
                # v via stationary swap: out[s_block, d] accumulated over k.
                # All 4 s-blocks share ONE psum bank; start=True clears
                # has_written for the whole bank, so only the very first
                # matmul may carry it — later blocks' first writes overwrite
                # (bit clear) then accumulate, which is exactly right.
                for sb in range(4):
                    nc.tensor.matmul(
                        psv[:, sb, :],
                        xs[:, sb * P:(sb + 1) * P],
                        wqkv_sb[:, k, 640:768],
                        start=(k == 0 and sb == 0),
                        stop=(k == N_KTILE - 1),
                        skip_group_check=True,
                    )

            # Fast psum eviction to bf16 on the ACT engine (bank-freeing order
            # = next chunk's write order: B, C, then the rotated q01 tag),
            # then all-bf16 RoPE on the DVE off the critical path.
            raw23 = tmp.tile([P, 2, SC], BF16, tag="r23", bufs=1, name=f"r23_{ci}")
            nc.scalar.copy(raw23[:], psq23[:, :, :])
            rawk = tmp.tile([P, SC], BF16, tag="rk", bufs=1, name=f"rk_{ci}")
            nc.scalar.copy(rawk[:], pskv[:, 0, :])
            nc.vector.tensor_copy(v_sb[:, ci * 4:(ci + 1) * 4, :], psv[:, :, :])
            raw01 = tmp.tile([P, 2, SC], BF16, tag="r01", bufs=1, name=f"r01_{ci}")
            nc.scalar.copy(raw01[:], psq01[:, :, :])

            for idx in (2, 3, 4, 0, 1):
                src = rawk[:] if idx == 4 \
                    else (raw01 if idx < 2 else raw23)[:, idx % 2, :]
                dst = kt_sb[:, s_lo:s_lo + SC] if idx == 4 \
                    else qt_sb[:, idx, s_lo:s_lo + SC]
                qc = tmp.tile([P, SC], BF16, tag="rqc", bufs=2, name=f"rq{ci}_{idx}")
                nc.vector.tensor_mul(qc[:], src, cos_sb[:, s_lo:s_lo + SC])
                qsw = tmp.tile([P, SC], BF16, tag="rqs", bufs=2, name=f"rs{ci}_{idx}")
                nc.vector.stream_shuffle(qsw[:], src, SWAP16)
                nc.vector.tensor_mul(qsw[:], qsw[:], sin_sb[:, s_lo:s_lo + SC])
                nc.vector.tensor_add(dst, qc[:], qsw[:])

        # ---- attention: one i-chunk-pair, head pairs pipelined ----------
        def attn(cp):
            n_j = 8 * (cp + 1)
            state = {}          # per pair: pss/pt/acc handles

            def u_list(t):
                return [u for u in (0, 1) if t < 8 * cp + 4 + 4 * u]

            def emit_scores(pr, t):
                ul = u_list(t)
                sd = state.setdefault(pr, {"pss": {}, "pt": {}, "acc": {}})
                for hh in range(2):
                    h = 2 * pr + hh
                    pss = ps.tile([P, IC], F32, tag=("A" if hh == 0 else "B"),
                                  name=f"pss{h}_{cp}_{t}")
                    sd["pss"][hh] = pss
                    for u in ul:
                        nc.tensor.matmul(
                            pss[:, u * SC:(u + 1) * SC],
                            kt_sb[:, t * P:(t + 1) * P],
                            qt_sb[:, h, cp * IC + u * SC:cp * IC + (u + 1) * SC],
                            start=True, stop=True,
                        )
                for hh in range(2):
                    h = 2 * pr + hh
                    pss = sd["pss"][hh]
                    pt = tmp.tile([P, IC], BF16, tag=f"pt{hh}", bufs=2,
                                  name=f"pt{h}_{cp}_{t}")
                    sd["pt"][(t, hh)] = pt
                    d = t - 8 * cp
                    if d < 0:
                        nc.scalar.activation(pt[:], pss[:], AF.Exp)
                    else:
                        for u in ul:
                            sl = slice(u * SC, (u + 1) * SC)
                            if t <= 8 * cp + 4 * u - 1:
                                nc.scalar.activation(pt[:, sl], pss[:, sl], AF.Exp)
                            else:
                                ms = tmp.tile([P, SC], F32, tag="ms", bufs=2,
                                              name=f"ms{h}_{cp}_{t}_{u}")
                                nc.vector.tensor_add(
                                    ms[:], pss[:, sl],
                                    mask_sb[:, IC - P * d + u * SC:
                                            2 * IC - P * d + (u - 1) * SC],
                                )
                                nc.scalar.activation(pt[:, sl], ms[:], AF.Exp)
                    # running bf16 accumulation of exp tiles (denominator)
                    if t == 0:
                        acc = tmp.tile([P, IC], BF16, tag=f"acc{hh}", bufs=2,
                                       name=f"acc{h}_{cp}")
                        nc.vector.tensor_copy(acc[:], pt[:])
                        sd["acc"][hh] = acc
                    else:
                        acc = sd["acc"][hh]
                        if len(ul) == 2:
                            nc.vector.tensor_add(acc[:], acc[:], pt[:])
                        else:
                            sl = slice(ul[0] * SC, (ul[0] + 1) * SC)
                            nc.vector.tensor_add(acc[:, sl], acc[:, sl], pt[:, sl])

            def emit_psy(pr, t):
                ul = u_list(t)
                sd = state[pr]
                for hh in range(2):
                    h = 2 * pr + hh
                    if t == 0:
                        sd[f"psy{hh}"] = ps.tile(
                            [P, IC], F32, tag=("C" if hh == 0 else "D"),
                            name=f"psy{h}_{cp}",
                        )
                    psy = sd[f"psy{hh}"]
                    pt = sd["pt"].pop((t, hh))
                    for u in ul:
                        sl = slice(u * SC, (u + 1) * SC)
                        nc.tensor.matmul(
                            psy[:, sl], v_sb[:, t, :], pt[:, sl],
                            start=(t == 0), stop=(t == 8 * cp + 3 + 4 * u),
                        )

            def emit_drain(pr):
                sd = state[pr]
                # evict psy fast so the next pair's scores can retake banks
                ysbs = {}
                for hh in range(2):
                    ysb = tmp.tile([P, IC], BF16, tag=f"ysb{hh}", bufs=1,
                                   name=f"ysb{2*pr+hh}_{cp}")
                    nc.scalar.copy(ysb[:], sd[f"psy{hh}"][:])
                    ysbs[hh] = ysb
                # denominator: one ones-matmul per (head, half); reciprocal
                # reads the psum directly (no SBUF staging)
                dtags = [("A", 0, 0), ("B", 0, 1), ("C", 1, 0), ("D", 1, 1)]
                rc1s = [
                    tmp.tile([1, IC], F32, tag=f"rc1{hh}", bufs=1,
                             name=f"rc1{pr}_{cp}_{hh}")
                    for hh in range(2)
                ]
                for tg, hh, u in dtags:
                    psd = ps.tile([1, SC], F32, tag=tg,
                                  name=f"psd{pr}_{cp}_{hh}_{u}")
                    nc.tensor.matmul(
                        psd[:], ones[:, 0:1],
                        sd["acc"][hh][:, u * SC:(u + 1) * SC],
                        start=True, stop=True,
                    )
                    nc.vector.reciprocal_approx_fast(
                        rc1s[hh][:, u * SC:(u + 1) * SC], psd[:]
                    )
                for hh in range(2):
                    h = 2 * pr + hh
                    rc1 = rc1s[hh]
                    rcb = tmp.tile([1, IC], BF16, tag=f"rcb{hh}", bufs=1,
                                   name=f"rcb{pr}_{cp}_{hh}")
                    nc.vector.tensor_copy(rcb[:], rc1[:])
                    rbb = tmp.tile([P, IC], BF16, tag=f"rbb{hh}", bufs=1,
                                   name=f"rbb{h}_{cp}")
                    nc.gpsimd.partition_broadcast(rbb[:], rcb[:])
                    yp = tmp.tile([P, IC], BF16, tag=f"yp{hh}", bufs=1,
                                  name=f"yp{h}_{cp}")
                    nc.vector.tensor_mul(yp[:], ysbs[hh][:], rbb[:])
                    nc.gpsimd.dma_start(
                        ybounce[cp][pr][hh * P:(hh + 1) * P, :], yp[:]
                    )
                    if debug_taps:
                        s_lo = cp * IC
                        nc.sync.dma_start(taps["yl"][:, h, s_lo:s_lo + IC], yp[:])
                        # dd tap holds 1/D (reciprocal) in this version
                        nc.sync.dma_start(
                            taps["dd"][h:h + 1, s_lo:s_lo + IC], rc1[:]
                        )
                nc.gpsimd.collective_compute(
                    "AllGather",
                    mybir.AluOpType.bypass,
                    replica_groups=[list(range(N_CORES))],
                    ins=[ybounce[cp][pr][:]],
                    outs=[ygather[cp][pr][:]],
                )
                del state[pr]

            items = [(pr, t) for pr in range(2) for t in range(n_j)]
            for i in range(len(items) + 1):
                if i < len(items):
                    emit_scores(*items[i])
                if i > 0:
                    pr2, t2 = items[i - 1]
                    emit_psy(pr2, t2)
                    if t2 == n_j - 1:
                        emit_drain(pr2)

        # ---- yg prefetch + output projection ----------------------------
        def load_yg(cp, pool, eng2):
            # queue choice matters: the trigger waits on the AllGather, and
            # everything behind it on the same queue stalls — so only queues
            # idle in the consuming phase may carry these.
            for pr in range(2):
                yg = pool.tile([P, 16, IC], BF16, tag=f"yg{cp}_{pr}",
                               name=f"yg{cp}_{pr}")
                src = ygather[cp][pr][:].rearrange("(b p) m -> p b m", p=P)
                nc.sync.dma_start(yg[:, 0:8, :], src[:, 0:8, :])
                eng2.dma_start(yg[:, 8:16, :], src[:, 8:16, :])
                yg_tiles[(cp, pr)] = yg

        def outproj(ci):
            g_lo = ci * SC
            cp, u = ci // 2, ci % 2
            for st_i in range(4):
                pso = ps.tile(
                    [P, SC], F32, tag=("C" if st_i % 2 == 0 else "D"),
                    name=f"pso{ci}_{st_i}",
                )
                for kt in range(N_KTILE):
                    c, h = kt // 4, kt % 4
                    src_t = yg_tiles[(cp, h // 2)]
                    nc.tensor.matmul(
                        pso[:],
                        src_t[:, 2 * c + h % 2,
                              u * SC + st_i * P:u * SC + (st_i + 1) * P],
                        wo_sb[:, kt, :],
                        start=(kt == 0), stop=(kt == N_KTILE - 1),
                    )
                ob = fin.tile([P, SC], F32, tag="ob", bufs=2,
                              name=f"ob{ci}_{st_i}")
                nc.vector.tensor_copy(ob[:], pso[:])
                nc.gpsimd.dma_start(
                    out[g_lo + st_i * P:g_lo + (st_i + 1) * P, :], ob[:]
                )

        # ---- pipeline ----------------------------------------------------
        proj(0)
        proj(1)
        attn(0)          # -> AG pair 0a, 0b
        nc.scalar.dma_start(wo_sb[:], woT.rearrange("(t p) m -> p t m", p=P))
        proj(2)
        proj(3)
        pw.release()
        pyg0 = tc.alloc_tile_pool(name="pyg0", bufs=1, side="right")
        load_yg(0, pyg0, nc.gpsimd)   # runs during attn(1); AG0 long done
        attn(1)          # -> AG pair 1a, 1b

        if debug_taps:
            nc.gpsimd.dma_start(taps["qt"][:], qt_sb[:])
            nc.gpsimd.dma_start(taps["kt"][:], kt_sb[:])
            nc.gpsimd.dma_start(taps["vv"][:], v_sb[:])

        tmp.release()
        stream.release()
        main.release()
        const.release()
        pyg1 = tc.alloc_tile_pool(name="pyg1", bufs=1, side="right")
        fin = tc.alloc_tile_pool(name="fin", bufs=1, side="right")
        load_yg(1, pyg1, nc.scalar)   # waits on AG1a/b, runs during outproj(0..1)
        outproj(0)
        outproj(1)
        outproj(2)
        outproj(3)

        for pool in (fin, pyg1, pyg0, pw2, dram, ps):
            pool.release()

    nc.compile()
    return nc


# ---------------------------------------------------------------------------
# host-side prep / unshard
# ---------------------------------------------------------------------------

def _perm128():
    """head-dim permutation: pair i=(16q+j) -> even at 32q+j, odd at 32q+16+j."""
    order = np.empty(128, dtype=np.int64)
    for i in range(64):
        q, j = i // 16, i % 16
        order[32 * q + j] = 2 * i
        order[32 * q + 16 + j] = 2 * i + 1
    return order


def _host_prep(x, freqs_cis, wq, wk, wv, wo):
    order = _perm128()
    xT = np.ascontiguousarray(x[0].T)                       # [DIM, S]
    scale = np.float32(1.0 / np.sqrt(HEAD_DIM))

    cosT = np.ascontiguousarray(freqs_cis[:, :, 0].T)       # [64, S]
    sinT = np.ascontiguousarray(freqs_cis[:, :, 1].T)
    cosd = np.empty((P, S), dtype=np.float32)
    sins = np.empty((P, S), dtype=np.float32)
    # converted to bf16 at the end of this function
    for q in range(4):
        cosd[32 * q:32 * q + 16] = cosT[16 * q:16 * q + 16]
        cosd[32 * q + 16:32 * q + 32] = cosT[16 * q:16 * q + 16]
        sins[32 * q:32 * q + 16] = -sinT[16 * q:16 * q + 16]
        sins[32 * q + 16:32 * q + 32] = sinT[16 * q:16 * q + 16]

    vv = np.arange(2 * IC)[None, :]
    jj = np.arange(P)[:, None]
    cmask = np.where(vv - IC >= jj, np.float32(0.0), np.float32(-1e9))
    cmask = np.ascontiguousarray(cmask, dtype=np.float32)

    xT16 = xT.astype(ml_dtypes.bfloat16)
    in_maps = []
    for c in range(N_CORES):
        wq_c = wq[c * 512:(c + 1) * 512].reshape(HPC, 128, DIM)[:, order, :]
        wq_c = (wq_c.reshape(512, DIM) * scale).astype(np.float32)
        wk_c = wk[c * 128:(c + 1) * 128][order]
        wv_c = wv[c * 128:(c + 1) * 128]
        wqkv_c = np.concatenate([wq_c, wk_c, wv_c], axis=0)
        wo_c = wo[c * 512:(c + 1) * 512]
        in_maps.append({
            "xT": xT16,
            "wqkvT": np.ascontiguousarray(wqkv_c.T).astype(ml_dtypes.bfloat16),
            "woT": np.ascontiguousarray(wo_c.T).astype(ml_dtypes.bfloat16),
            "cosd": cosd.astype(ml_dtypes.bfloat16),
            "sins": sins.astype(ml_dtypes.bfloat16),
            "cmask": cmask,
        })
    return in_maps


_NC_CACHE = {}


def get_nc(debug_taps=False):
    key = bool(debug_taps)
    if key not in _NC_CACHE:
        _NC_CACHE[key] = build(debug_taps=key)
    return _NC_CACHE[key]


def kernel(x, freqs_cis, mask, wq, wk, wv, wo, _trace=False, _debug_taps=False,
           _warmup=False):
    in_maps = _host_prep(x, freqs_cis, wq, wk, wv, wo)
    nc = get_nc(_debug_taps)
    if _warmup:
        run_bass_kernel_spmd(
            nc, in_maps, core_ids=list(range(N_CORES)), trace=False
        )
    res = run_bass_kernel_spmd(
        nc, in_maps, core_ids=list(range(N_CORES)), trace=_trace
    )
    full = np.concatenate([res.results[c]["out"] for c in range(N_CORES)], axis=1)
    out = full.reshape(1, S, DIM).astype(np.float32)
    if _trace or _debug_taps:
        kernel.last_results = res
    return out
